# revision 9
# baseline (speedup 1.0000x reference)
"""Sparse transposed-conv (27-tap gather-GEMM) + BatchNorm + LeakyReLU on 8 TRN2 cores.

V2 strategy (voxel-sharded, all-SBUF h, transposed class-gather, stationary-swap GEMM):
  - x stored bf16 in HBM. Host groups valid (j -> i) pairs per (tap k, class
    p = j%8); device gathers channel-major via transposed dma_gather (elem =
    256B = rows j, j+1), idx = j>>3 (int16 covers all 200k rows at stride
    1024B); one gather call per (strip, class) on 4 SWDGE queues.
  - GEMM: gathered chunk [128 = 2x64ch, 128 slots] is the matmul STATIONARY
    (lhsT); rhs = [[Wk],[0]] kills the j+1 garbage half; output lands
    voxel-major [128 slots, 64ch] in PSUM. Zero transposes anywhere.
  - MM chunks iterate in scatter order so each PSUM bank maps to one
    contiguous scatter range (single PSUM->SBUF copy per bank).
  - h is SBUF-resident: 2 ping-pong pairs of parity-split bf16 buffers fed by
    SBUF-destination dma_scatter_add (no HBM round trips). Center tap is
    dense: host-pretransposed xcT chunks as stationary; its copies initialize
    pair A (incl. zero rows 25000..25087).
  - BN: per-partition sum/sumsq -> AllReduce [128,128] over 8 cores ->
    partition_all_reduce -> coefficients; apply = DVE mul/add + ACT LeakyReLU
    (bf16), y written via gpsimd cast-DMA to fp32.
"""
import numpy as np

import concourse.bass as bass
import concourse.bass_isa as bass_isa
import concourse.mybir as mybir
import concourse.bacc as bacc
import concourse.tile as tile
from concourse import bass_utils

N = 200000
C = 8
V = N // C          # 25000
D = 64
K = 27
KC = 13             # center tap
NCLS = 8
VB = 196            # valid 128-row blocks (rows 0..25087; 25000.. are zeros)
HG = 100            # g-slots per parity buffer (g 98,99 = trash)
TRASH0 = 25088
EPS = 1e-5
NEG = 0.01
F32 = mybir.dt.float32
BF16 = mybir.dt.bfloat16
I16 = mybir.dt.int16

TAPS = [k for k in range(K) if k != KC]
STRIPS = [TAPS[i:i + 3] for i in range(0, 26, 3)]   # 8x3 + 1x2


def _r128(n):
    return (n + 127) & ~127


def _pack16_into(slab, col0, vals):
    w = vals.reshape(-1, 16).T
    L16 = w.shape[1]
    for r in range(8):
        slab[r * 16:(r + 1) * 16, col0:col0 + L16] = w
    return col0 + L16


def _prep_host(nbr):
    nbr = np.asarray(nbr, np.int64)
    lists = {}
    for c in range(C):
        sl = slice(c * V, (c + 1) * V)
        for k in TAPS:
            src = nbr[k, sl]
            vi = np.nonzero(src >= 0)[0]
            j = src[vi]
            o = np.argsort(j, kind="stable")
            j, vi = j[o], vi[o]
            p = j & 7
            for cls in range(NCLS):
                m = p == cls
                lists[(c, k, cls)] = ((j[m] >> 3).astype(np.int16),
                                      vi[m].astype(np.int16))
    seg_len = {}
    for k in TAPS:
        for cls in range(NCLS):
            mx = max(len(lists[(c, k, cls)][0]) for c in range(C))
            seg_len[(k, cls)] = _r128(mx) if mx else 0

    plan = []
    goff = 0
    for taps in STRIPS:
        cls_lens = [sum(seg_len[(k, cls)] for k in taps) for cls in range(NCLS)]
        SG = sum(cls_lens)
        s_chunk_off = {}
        off = 0
        for k in taps:
            s_chunk_off[k] = off
            off += sum(seg_len[(k, cls)] for cls in range(NCLS)) // 128
        g_seg_off = {}
        off = 0
        for cls in range(NCLS):
            for k in taps:
                g_seg_off[(k, cls)] = off
                off += seg_len[(k, cls)]
        chunks = []   # in scatter order: (gather_chunk, tap, scatter_chunk)
        for k in taps:
            sc = s_chunk_off[k]
            for cls in range(NCLS):
                for t in range(seg_len[(k, cls)] // 128):
                    chunks.append((g_seg_off[(k, cls)] // 128 + t, k, sc))
                    sc += 1
        tap_ranges = {}
        for k in taps:
            lo = s_chunk_off[k]
            hi = lo + sum(seg_len[(k, cls)] for cls in range(NCLS)) // 128
            tap_ranges[k] = (lo, hi)
        plan.append(dict(taps=taps, g_off=goff, SG=SG, cls_lens=cls_lens,
                         chunks=chunks, tap_ranges=tap_ranges))
        goff += SG
    GT = goff

    gslab = np.zeros((C, 128, GT // 16), np.int16)
    sslab = np.zeros((C, 128, GT // 16), np.int16)
    for c in range(C):
        for st, taps in zip(plan, [s["taps"] for s in plan]):
            col = st["g_off"] // 16
            for cls in range(NCLS):
                for k in st["taps"]:
                    L = seg_len[(k, cls)]
                    if L == 0:
                        continue
                    g, _ = lists[(c, k, cls)]
                    gp = np.zeros(L, np.int16)
                    gp[:len(g)] = g
                    col = _pack16_into(gslab[c], col, gp)
            col = st["g_off"] // 16
            for k in st["taps"]:
                for cls in range(NCLS):
                    L = seg_len[(k, cls)]
                    if L == 0:
                        continue
                    _, s = lists[(c, k, cls)]
                    sp = np.empty(L, np.int16)
                    pad = np.arange(len(s), L, dtype=np.int16)
                    sp[:len(s)] = s
                    sp[len(s):] = TRASH0 + (pad & 511)
                    col = _pack16_into(sslab[c], col, sp)
    return plan, GT, gslab, sslab


def _build_body(nc, plan, GT):
    import os as _os
    PH = int(_os.environ.get("V2PHASE", "5"))
    x_d = nc.dram_tensor("x_d", [N + 8, D], BF16, kind="ExternalInput")
    xct_d = nc.dram_tensor("xct_d", [D, VB * 128], BF16, kind="ExternalInput")
    w2_d = nc.dram_tensor("w2_d", [128, K * D], BF16, kind="ExternalInput")
    gam_d = nc.dram_tensor("gam_d", [128, D], F32, kind="ExternalInput")
    bet_d = nc.dram_tensor("bet_d", [128, D], F32, kind="ExternalInput")
    gi_d = nc.dram_tensor("gi_d", [128, GT // 16], I16, kind="ExternalInput")
    si_d = nc.dram_tensor("si_d", [128, GT // 16], I16, kind="ExternalInput")
    y_d = nc.dram_tensor("y_d", [V, D], F32, kind="ExternalOutput")

    sq = [0]

    def nextsq():
        # scatter queues rotate 1..3; queue 0 is reserved for transposed
        # gathers (concurrent transposed gathers on different queues race on
        # the shared xbar -> corruption; same-queue ring order is safe).
        q = 1 + sq[0] % 3
        sq[0] += 1
        return q

    with tile.TileContext(nc) as tc:
        with tc.tile_pool(name="sb", bufs=1) as sb, \
             tc.tile_pool(name="gio", bufs=2) as gio, \
             tc.tile_pool(name="sio", bufs=2) as sio, \
             tc.tile_pool(name="cio", bufs=2) as cio, \
             tc.tile_pool(name="ps", bufs=4, space="PSUM") as ps, \
             tc.tile_pool(name="dram", bufs=1, space="DRAM") as dram:

            gi_t = sb.tile([128, GT // 16], I16)
            nc.sync.dma_start(gi_t[:], gi_d[:, :])
            si_t = sb.tile([128, GT // 16], I16)
            nc.sync.dma_start(si_t[:], si_d[:, :])
            w2_t = sb.tile([128, K * D], BF16)
            nc.sync.dma_start(w2_t[:], w2_d[:, :])

            # h: two DRAM f32 ping-pong accumulators (rows 25088.. = trash)
            h_a = nc.dram_tensor("h_a", [25600, D], F32, kind="Internal")
            h_b = nc.dram_tensor("h_b", [25600, D], F32, kind="Internal")
            zt = sb.tile([128, 28, D], F32)
            nc.vector.memset(zt[:].rearrange("p a b -> p (a b)"), 0.0)
            hbv = h_b[0:25088, :].rearrange("(m p) c -> p m c", p=128)
            for z0 in range(0, VB, 28):
                nc.sync.dma_start(hbv[:, z0:z0 + 28, :], zt[:, :, :])

            # ---- center tap: 16-block batches -> staging -> h_a ----
            hav = h_a[0:25088, :].rearrange("(m p) c -> p m c", p=128)
            for b0 in range(0, VB, 16):
                nb = min(16, VB - b0)
                xc = cio.tile([64, 16 * 128], BF16, tag="xc")
                nc.sync.dma_start(xc[:, 0:nb * 128],
                                  xct_d[:, b0 * 128:(b0 + nb) * 128])
                stg = cio.tile([128, 16, D], F32, tag="stg")
                for h0 in range(0, nb, 8):
                    nh = min(8, nb - h0)
                    pa = ps.tile([128, 512], F32, tag="cps", space="PSUM")
                    for j in range(nh):
                        nc.tensor.matmul(
                            out=pa[:, j * 64:(j + 1) * 64],
                            lhsT=xc[:, (h0 + j) * 128:(h0 + j + 1) * 128],
                            rhs=w2_t[0:64, KC * D:KC * D + D],
                            start=True, stop=True)
                    if h0 == 0:
                        nc.scalar.activation(
                            stg[:, 0:nh, :],
                            pa[:, 0:nh * 64].rearrange("p (a b) -> p a b", b=64),
                            mybir.ActivationFunctionType.Copy, bias=0.0)
                    else:
                        nc.vector.tensor_copy(
                            stg[:, h0:h0 + nh, :],
                            pa[:, 0:nh * 64].rearrange("p (a b) -> p a b", b=64))
                nc.sync.dma_start(hav[:, b0:b0 + nb, :], stg[:, 0:nb, :])

            # ---- sparse taps: strip pipeline ----
            scat_ord = [0]
            for st in (plan if PH >= 2 else []):
                SG = st["SG"]
                gb = gio.tile([128, 1, SG], BF16, tag="gb")
                off = 0
                for cls in range(NCLS):
                    Lc = st["cls_lens"][cls]
                    if Lc == 0:
                        continue
                    xv = x_d[cls:cls + N, :].rearrange(
                        "(a b) c -> a (b c)", b=8)
                    nc.gpsimd.dma_gather(
                        out_ap=gb[:, :, off:off + Lc],
                        in_ap=xv[:, 0:128],
                        idxs_ap=gi_t[:, (st["g_off"] + off) // 16:
                                     (st["g_off"] + off + Lc) // 16],
                        num_idxs=Lc, num_idxs_reg=Lc,
                        elem_size=128, elem_step=512,
                        transpose=True, single_packet=False,
                        queue_num=0)
                    off += Lc
                sk = sio.tile([128, SG // 128, D], F32, tag="sk")
                chunks = st["chunks"]
                if PH < 3:
                    continue
                for c0 in range(0, len(chunks), 8):
                    grp = chunks[c0:c0 + 8]
                    pa = ps.tile([128, 512], F32, tag="sps", space="PSUM")
                    for j, (gch, k, sch) in enumerate(grp):
                        nc.tensor.matmul(
                            out=pa[:, j * 64:(j + 1) * 64],
                            lhsT=gb[:, 0, gch * 128:(gch + 1) * 128],
                            rhs=w2_t[:, k * D:k * D + D],
                            start=True, stop=True)
                    sc0 = grp[0][2]
                    if (c0 // 8) % 2 == 0:
                        nc.scalar.activation(
                            sk[:, sc0:sc0 + len(grp), :],
                            pa[:, 0:len(grp) * 64].rearrange(
                                "p (a b) -> p a b", b=64),
                            mybir.ActivationFunctionType.Copy, bias=0.0)
                    else:
                        nc.vector.tensor_copy(
                            sk[:, sc0:sc0 + len(grp), :],
                            pa[:, 0:len(grp) * 64].rearrange(
                                "p (a b) -> p a b", b=64))
                for k in (st["taps"] if PH >= 4 else []):
                    lo, hi = st["tap_ranges"][k]
                    pair = scat_ord[0] % 2
                    scat_ord[0] += 1
                    nc.gpsimd.dma_scatter_add(
                        out_ap=(h_a if pair == 0 else h_b)[:, :],
                        in_ap=sk[:, lo:hi, :],
                        idxs_ap=si_t[:, (st["g_off"] + lo * 128) // 16:
                                     (st["g_off"] + hi * 128) // 16],
                        num_idxs=(hi - lo) * 128,
                        num_idxs_reg=(hi - lo) * 128,
                        elem_size=D,
                        single_packet=False,
                        queue_num=nextsq())

            if PH < 5:
                dum = sb.tile([128, 1, D], F32)
                nc.vector.memset(dum[:].rearrange("p a b -> p (a b)"), 0.0)
                nc.sync.dma_start(
                    y_d[0:128, :].rearrange("(o p) c -> p o c", o=1), dum[:])
                return
            # ---- BN stats: stream h_a+h_b, keep sum resident ----
            u = sb.tile([128, VB, D], F32)
            scr = sb.tile([128, 16, D], F32)
            acc2 = sb.tile([128, 2 * 13, D], F32)
            sacc = sb.tile([128, 2 * D], F32)
            hav2 = h_a[0:25088, :].rearrange("(m p) c -> p m c", p=128)
            hbv2 = h_b[0:25088, :].rearrange("(m p) c -> p m c", p=128)
            for ci, b0 in enumerate(range(0, VB, 16)):
                nb = min(16, VB - b0)
                ta = cio.tile([128, 16, D], F32, tag="ta")
                nc.sync.dma_start(ta[:, 0:nb, :], hav2[:, b0:b0 + nb, :])
                tb = cio.tile([128, 16, D], F32, tag="tb")
                nc.sync.dma_start(tb[:, 0:nb, :], hbv2[:, b0:b0 + nb, :])
                nc.vector.tensor_tensor(
                    out=u[:, b0:b0 + nb, :], in0=ta[:, 0:nb, :],
                    in1=tb[:, 0:nb, :], op=mybir.AluOpType.add)
                nc.vector.tensor_reduce(
                    out=acc2[:, 2 * ci, :],
                    in_=u[:, b0:b0 + nb, :].transpose([0, 2, 1]),
                    axis=mybir.AxisListType.X, op=mybir.AluOpType.add)
                nc.vector.tensor_tensor(
                    out=scr[:, 0:nb, :], in0=u[:, b0:b0 + nb, :],
                    in1=u[:, b0:b0 + nb, :], op=mybir.AluOpType.mult)
                nc.vector.tensor_reduce(
                    out=acc2[:, 2 * ci + 1, :],
                    in_=scr[:, 0:nb, :].transpose([0, 2, 1]),
                    axis=mybir.AxisListType.X, op=mybir.AluOpType.add)
            nc.vector.tensor_reduce(
                out=sacc[:, 0:D],
                in_=acc2[:, 0:26:2, :].transpose([0, 2, 1]),
                axis=mybir.AxisListType.X, op=mybir.AluOpType.add)
            nc.vector.tensor_reduce(
                out=sacc[:, D:2 * D],
                in_=acc2[:, 1:26:2, :].transpose([0, 2, 1]),
                axis=mybir.AxisListType.X, op=mybir.AluOpType.add)

            cc_in = dram.tile([128, 2 * D], F32)
            cc_out = dram.tile([128, 2 * D], F32)
            nc.gpsimd.dma_start(cc_in[:], sacc[:])
            import os as _os
            if _os.environ.get("V2NOCC"):
                nc.gpsimd.dma_start(cc_out[:], cc_in[:])
            else:
                nc.gpsimd.collective_compute(
                    "AllReduce", mybir.AluOpType.add,
                    replica_groups=[list(range(C))],
                    ins=[cc_in.opt()], outs=[cc_out.opt()])
            tot = sb.tile([128, 2 * D], F32)
            nc.sync.dma_start(tot[:], cc_out[:])
            totr = sb.tile([128, 2 * D], F32)
            nc.gpsimd.partition_all_reduce(totr[:], tot[:], 128,
                                           bass_isa.ReduceOp.add)

            me = sb.tile([128, 2 * D], F32)
            nc.vector.tensor_scalar_mul(me[:], totr[:], 1.0 / N)
            var = sb.tile([128, D], F32)
            nc.vector.tensor_tensor(out=var[:], in0=me[:, 0:D],
                                    in1=me[:, 0:D], op=mybir.AluOpType.mult)
            nc.vector.tensor_tensor(out=var[:], in0=me[:, D:2 * D],
                                    in1=var[:], op=mybir.AluOpType.subtract)
            eps_t = sb.tile([128, 1], F32)
            nc.gpsimd.memset(eps_t[:], EPS)
            std = sb.tile([128, D], F32)
            nc.scalar.activation(std[:], var[:],
                                 mybir.ActivationFunctionType.Sqrt,
                                 bias=eps_t[:])
            rstd = sb.tile([128, D], F32)
            nc.vector.reciprocal(rstd[:], std[:])
            gam = sb.tile([128, D], F32)
            nc.sync.dma_start(gam[:], gam_d[:, :])
            bet = sb.tile([128, D], F32)
            nc.sync.dma_start(bet[:], bet_d[:, :])
            sc_f = sb.tile([128, D], F32)
            nc.vector.tensor_tensor(out=sc_f[:], in0=rstd[:], in1=gam[:],
                                    op=mybir.AluOpType.mult)
            cb = sb.tile([128, D], F32)
            nc.vector.tensor_tensor(out=cb[:], in0=me[:, 0:D], in1=sc_f[:],
                                    op=mybir.AluOpType.mult)
            nc.vector.tensor_tensor(out=cb[:], in0=bet[:], in1=cb[:],
                                    op=mybir.AluOpType.subtract)

            # ---- apply + y write (chunked so DVE/ACT/DMA pipeline) ----
            # u holds h sum; y row = 128m + p, m = u block index.
            yv = y_d[0:24960, :].rearrange("(m p) c -> p m c", p=128)
            APC = 28
            for a0 in range(0, VB, APC):
                na = min(APC, VB - a0)
                sc_bc = sc_f[:, None, :].to_broadcast([128, na, D])
                cb_bc = cb[:, None, :].to_broadcast([128, na, D])
                nc.vector.tensor_tensor(out=u[:, a0:a0 + na, :],
                                        in0=u[:, a0:a0 + na, :],
                                        in1=sc_bc, op=mybir.AluOpType.mult)
                nc.vector.tensor_tensor(out=u[:, a0:a0 + na, :],
                                        in0=u[:, a0:a0 + na, :],
                                        in1=cb_bc, op=mybir.AluOpType.add)
                nc.scalar.activation(u[:, a0:a0 + na, :], u[:, a0:a0 + na, :],
                                     mybir.ActivationFunctionType.Lrelu,
                                     bias=0.0, alpha=NEG)
                hi = min(a0 + na, 195)
                if hi > a0:
                    nc.sync.dma_start(yv[:, a0:hi, :], u[:, a0:hi, :])
            nc.sync.dma_start(
                y_d[24960:25000, :].rearrange("(o p) c -> p o c", o=1),
                u[0:40, 195:196, :])


_CACHE = {}


def build(nbr):
    nbr = np.asarray(nbr)
    key = nbr.tobytes()[:4096] + nbr.tobytes()[-4096:]
    if key in _CACHE:
        return _CACHE[key]
    plan, GT, gslab, sslab = _prep_host(nbr)
    nc = bacc.Bacc("TRN2", target_bir_lowering=False, debug=False,
                   num_devices=C, num_swdge_queues=4)
    _build_body(nc, plan, GT)
    nc.compile()
    _CACHE[key] = (nc, gslab, sslab)
    return nc, gslab, sslab


def _to_bf16(a):
    import jax.numpy as jnp
    return np.asarray(jnp.asarray(a, jnp.bfloat16))


def make_in_maps(x, W, gamma, beta, gslab, sslab):
    xpad = np.zeros((N + 8, D), np.float32)
    xpad[:N] = x
    x_bf = _to_bf16(xpad)
    w2 = np.zeros((128, K * D), np.float32)
    for k in range(K):
        w2[0:D, k * D:(k + 1) * D] = W[k]
    w2_bf = _to_bf16(w2)
    gam_r = np.ascontiguousarray(np.broadcast_to(gamma, (128, D)))
    bet_r = np.ascontiguousarray(np.broadcast_to(beta, (128, D)))
    in_maps = []
    for c in range(C):
        xc = np.zeros((D, VB * 128), np.float32)
        xc[:, 0:V] = x[c * V:(c + 1) * V].T
        in_maps.append({
            "x_d": x_bf,
            "xct_d": _to_bf16(xc),
            "w2_d": w2_bf,
            "gam_d": gam_r,
            "bet_d": bet_r,
            "gi_d": gslab[c],
            "si_d": sslab[c],
        })
    return in_maps


def kernel(x, W, gamma, beta, nbr):
    x = np.ascontiguousarray(np.asarray(x, np.float32))
    W = np.ascontiguousarray(np.asarray(W, np.float32))
    gamma = np.asarray(gamma, np.float32).reshape(D)
    beta = np.asarray(beta, np.float32).reshape(D)
    nbr = np.asarray(nbr)
    nc, gslab, sslab = build(nbr)
    in_maps = make_in_maps(x, W, gamma, beta, gslab, sslab)
    res = bass_utils.run_bass_kernel_spmd(nc, in_maps, core_ids=list(range(C)))
    return np.concatenate([res.results[c]["y_d"] for c in range(C)], axis=0)


# revision 10
# speedup vs baseline: 1.4623x; 1.4623x over previous
"""Sparse transposed-conv (27-tap gather-GEMM) + BatchNorm + LeakyReLU on 8 TRN2 cores.

V2 strategy (voxel-sharded, all-SBUF h, transposed class-gather, stationary-swap GEMM):
  - x stored bf16 in HBM. Host groups valid (j -> i) pairs per (tap k, class
    p = j%8); device gathers channel-major via transposed dma_gather (elem =
    256B = rows j, j+1), idx = j>>3 (int16 covers all 200k rows at stride
    1024B); one gather call per (strip, class) on 4 SWDGE queues.
  - GEMM: gathered chunk [128 = 2x64ch, 128 slots] is the matmul STATIONARY
    (lhsT); rhs = [[Wk],[0]] kills the j+1 garbage half; output lands
    voxel-major [128 slots, 64ch] in PSUM. Zero transposes anywhere.
  - MM chunks iterate in scatter order so each PSUM bank maps to one
    contiguous scatter range (single PSUM->SBUF copy per bank).
  - h is SBUF-resident: 2 ping-pong pairs of parity-split bf16 buffers fed by
    SBUF-destination dma_scatter_add (no HBM round trips). Center tap is
    dense: host-pretransposed xcT chunks as stationary; its copies initialize
    pair A (incl. zero rows 25000..25087).
  - BN: per-partition sum/sumsq -> AllReduce [128,128] over 8 cores ->
    partition_all_reduce -> coefficients; apply = DVE mul/add + ACT LeakyReLU
    (bf16), y written via gpsimd cast-DMA to fp32.
"""
import numpy as np

import concourse.bass as bass
import concourse.bass_isa as bass_isa
import concourse.mybir as mybir
import concourse.bacc as bacc
import concourse.tile as tile
from concourse import bass_utils

N = 200000
C = 8
V = N // C          # 25000
D = 64
K = 27
KC = 13             # center tap
NCLS = 8
VB = 196            # valid 128-row blocks (rows 0..25087; 25000.. are zeros)
HG = 100            # g-slots per parity buffer (g 98,99 = trash)
TRASH0 = 25088
EPS = 1e-5
NEG = 0.01
F32 = mybir.dt.float32
BF16 = mybir.dt.bfloat16
I16 = mybir.dt.int16

TAPS = [k for k in range(K) if k != KC]
STRIPS = [TAPS[i:i + 3] for i in range(0, 26, 3)]   # 8x3 + 1x2


def _r128(n):
    return (n + 127) & ~127


def _pack16_into(slab, col0, vals):
    w = vals.reshape(-1, 16).T
    L16 = w.shape[1]
    for r in range(8):
        slab[r * 16:(r + 1) * 16, col0:col0 + L16] = w
    return col0 + L16


def _prep_host(nbr):
    nbr = np.asarray(nbr, np.int64)
    lists = {}
    for c in range(C):
        sl = slice(c * V, (c + 1) * V)
        for k in TAPS:
            src = nbr[k, sl]
            vi = np.nonzero(src >= 0)[0]
            j = src[vi]
            o = np.argsort(j, kind="stable")
            j, vi = j[o], vi[o]
            p = j & 7
            for cls in range(NCLS):
                m = p == cls
                lists[(c, k, cls)] = ((j[m] >> 3).astype(np.int16),
                                      vi[m].astype(np.int16))
    seg_len = {}
    for k in TAPS:
        for cls in range(NCLS):
            mx = max(len(lists[(c, k, cls)][0]) for c in range(C))
            seg_len[(k, cls)] = _r128(mx) if mx else 0

    plan = []
    goff = 0
    for taps in STRIPS:
        cls_lens = [sum(seg_len[(k, cls)] for k in taps) for cls in range(NCLS)]
        SG = sum(cls_lens)
        s_chunk_off = {}
        off = 0
        for k in taps:
            s_chunk_off[k] = off
            off += sum(seg_len[(k, cls)] for cls in range(NCLS)) // 128
        g_seg_off = {}
        off = 0
        for cls in range(NCLS):
            for k in taps:
                g_seg_off[(k, cls)] = off
                off += seg_len[(k, cls)]
        chunks = []   # in scatter order: (gather_chunk, tap, scatter_chunk)
        for k in taps:
            sc = s_chunk_off[k]
            for cls in range(NCLS):
                for t in range(seg_len[(k, cls)] // 128):
                    chunks.append((g_seg_off[(k, cls)] // 128 + t, k, sc))
                    sc += 1
        tap_ranges = {}
        for k in taps:
            lo = s_chunk_off[k]
            hi = lo + sum(seg_len[(k, cls)] for cls in range(NCLS)) // 128
            tap_ranges[k] = (lo, hi)
        plan.append(dict(taps=taps, g_off=goff, SG=SG, cls_lens=cls_lens,
                         chunks=chunks, tap_ranges=tap_ranges))
        goff += SG
    GT = goff

    gslab = np.zeros((C, 128, GT // 16), np.int16)
    sslab = np.zeros((C, 128, GT // 16), np.int16)
    for c in range(C):
        for st, taps in zip(plan, [s["taps"] for s in plan]):
            col = st["g_off"] // 16
            for cls in range(NCLS):
                for k in st["taps"]:
                    L = seg_len[(k, cls)]
                    if L == 0:
                        continue
                    g, _ = lists[(c, k, cls)]
                    gp = np.zeros(L, np.int16)
                    gp[:len(g)] = g
                    col = _pack16_into(gslab[c], col, gp)
            col = st["g_off"] // 16
            for k in st["taps"]:
                for cls in range(NCLS):
                    L = seg_len[(k, cls)]
                    if L == 0:
                        continue
                    _, s = lists[(c, k, cls)]
                    s32 = s.astype(np.int32)
                    sp = np.empty(L, np.int16)
                    pad = np.arange(len(s), L, dtype=np.int32)
                    # h layout: partition-major, row = (i%128)*200 + i//128
                    sp[:len(s)] = ((s32 & 127) * 200 + (s32 >> 7)).astype(np.int16)
                    # trash: blocks 196..199 of any partition
                    sp[len(s):] = ((pad & 127) * 200 + 196 + ((pad >> 7) & 3)).astype(np.int16)
                    col = _pack16_into(sslab[c], col, sp)
    return plan, GT, gslab, sslab


def _build_body(nc, plan, GT):
    import os as _os
    PH = int(_os.environ.get("V2PHASE", "5"))
    x_d = nc.dram_tensor("x_d", [N + 8, D], BF16, kind="ExternalInput")
    xct_d = nc.dram_tensor("xct_d", [D, VB * 128], BF16, kind="ExternalInput")
    w2_d = nc.dram_tensor("w2_d", [128, K * D], BF16, kind="ExternalInput")
    gam_d = nc.dram_tensor("gam_d", [128, D], F32, kind="ExternalInput")
    bet_d = nc.dram_tensor("bet_d", [128, D], F32, kind="ExternalInput")
    gi_d = nc.dram_tensor("gi_d", [128, GT // 16], I16, kind="ExternalInput")
    si_d = nc.dram_tensor("si_d", [128, GT // 16], I16, kind="ExternalInput")
    y_d = nc.dram_tensor("y_d", [V, D], F32, kind="ExternalOutput")

    sq = [0]

    def nextsq():
        # scatter queues rotate 1..3; queue 0 is reserved for transposed
        # gathers (concurrent transposed gathers on different queues race on
        # the shared xbar -> corruption; same-queue ring order is safe).
        q = 1 + sq[0] % 3
        sq[0] += 1
        return q

    with tile.TileContext(nc) as tc:
        with tc.tile_pool(name="sb", bufs=1) as sb, \
             tc.tile_pool(name="gio", bufs=2) as gio, \
             tc.tile_pool(name="sio", bufs=2) as sio, \
             tc.tile_pool(name="cio", bufs=2) as cio, \
             tc.tile_pool(name="ps", bufs=4, space="PSUM") as ps, \
             tc.tile_pool(name="dram", bufs=1, space="DRAM") as dram:

            gi_t = sb.tile([128, GT // 16], I16)
            nc.sync.dma_start(gi_t[:], gi_d[:, :])
            si_t = sb.tile([128, GT // 16], I16)
            nc.sync.dma_start(si_t[:], si_d[:, :])
            w2_t = sb.tile([128, K * D], BF16)
            nc.sync.dma_start(w2_t[:], w2_d[:, :])

            # h: two DRAM f32 ping-pong accumulators (rows 25088.. = trash)
            h_a = nc.dram_tensor("h_a", [25600, D], F32, kind="Internal")
            h_b = nc.dram_tensor("h_b", [25600, D], F32, kind="Internal")
            zt = sb.tile([128, 28, D], F32)
            nc.vector.memset(zt[:].rearrange("p a b -> p (a b)"), 0.0)
            hbv = h_b[:, :].rearrange("(p m) c -> p m c", p=128)
            for z0 in range(0, VB, 28):
                nc.sync.dma_start(hbv[:, z0:z0 + 28, :], zt[:, :, :])

            # ---- center tap: 16-block batches -> staging -> h_a ----
            hav = h_a[:, :].rearrange("(p m) c -> p m c", p=128)
            for b0 in range(0, VB, 16):
                nb = min(16, VB - b0)
                xc = cio.tile([64, 16 * 128], BF16, tag="xc")
                nc.sync.dma_start(xc[:, 0:nb * 128],
                                  xct_d[:, b0 * 128:(b0 + nb) * 128])
                stg = cio.tile([128, 16, D], F32, tag="stg")
                for h0 in range(0, nb, 8):
                    nh = min(8, nb - h0)
                    pa = ps.tile([128, 512], F32, tag="cps", space="PSUM")
                    for j in range(nh):
                        nc.tensor.matmul(
                            out=pa[:, j * 64:(j + 1) * 64],
                            lhsT=xc[:, (h0 + j) * 128:(h0 + j + 1) * 128],
                            rhs=w2_t[0:64, KC * D:KC * D + D],
                            start=True, stop=True)
                    if h0 == 0:
                        nc.scalar.activation(
                            stg[:, 0:nh, :],
                            pa[:, 0:nh * 64].rearrange("p (a b) -> p a b", b=64),
                            mybir.ActivationFunctionType.Copy, bias=0.0)
                    else:
                        nc.vector.tensor_copy(
                            stg[:, h0:h0 + nh, :],
                            pa[:, 0:nh * 64].rearrange("p (a b) -> p a b", b=64))
                nc.sync.dma_start(hav[:, b0:b0 + nb, :], stg[:, 0:nb, :])

            # ---- sparse taps: strip pipeline ----
            scat_ord = [0]
            for st in (plan if PH >= 2 else []):
                SG = st["SG"]
                gb = gio.tile([128, 1, SG], BF16, tag="gb")
                off = 0
                for cls in range(NCLS):
                    Lc = st["cls_lens"][cls]
                    if Lc == 0:
                        continue
                    xv = x_d[cls:cls + N, :].rearrange(
                        "(a b) c -> a (b c)", b=8)
                    nc.gpsimd.dma_gather(
                        out_ap=gb[:, :, off:off + Lc],
                        in_ap=xv[:, 0:128],
                        idxs_ap=gi_t[:, (st["g_off"] + off) // 16:
                                     (st["g_off"] + off + Lc) // 16],
                        num_idxs=Lc, num_idxs_reg=Lc,
                        elem_size=128, elem_step=512,
                        transpose=True, single_packet=False,
                        queue_num=0)
                    off += Lc
                sk = sio.tile([128, SG // 128, D], F32, tag="sk")
                chunks = st["chunks"]
                if PH < 3:
                    continue
                for c0 in range(0, len(chunks), 8):
                    grp = chunks[c0:c0 + 8]
                    pa = ps.tile([128, 512], F32, tag="sps", space="PSUM")
                    for j, (gch, k, sch) in enumerate(grp):
                        nc.tensor.matmul(
                            out=pa[:, j * 64:(j + 1) * 64],
                            lhsT=gb[:, 0, gch * 128:(gch + 1) * 128],
                            rhs=w2_t[:, k * D:k * D + D],
                            start=True, stop=True)
                    sc0 = grp[0][2]
                    if (c0 // 8) % 2 == 0:
                        nc.scalar.activation(
                            sk[:, sc0:sc0 + len(grp), :],
                            pa[:, 0:len(grp) * 64].rearrange(
                                "p (a b) -> p a b", b=64),
                            mybir.ActivationFunctionType.Copy, bias=0.0)
                    else:
                        nc.vector.tensor_copy(
                            sk[:, sc0:sc0 + len(grp), :],
                            pa[:, 0:len(grp) * 64].rearrange(
                                "p (a b) -> p a b", b=64))
                for k in (st["taps"] if PH >= 4 else []):
                    lo, hi = st["tap_ranges"][k]
                    pair = scat_ord[0] % 2
                    scat_ord[0] += 1
                    nc.gpsimd.dma_scatter_add(
                        out_ap=(h_a if pair == 0 else h_b)[:, :],
                        in_ap=sk[:, lo:hi, :],
                        idxs_ap=si_t[:, (st["g_off"] + lo * 128) // 16:
                                     (st["g_off"] + hi * 128) // 16],
                        num_idxs=(hi - lo) * 128,
                        num_idxs_reg=(hi - lo) * 128,
                        elem_size=D,
                        single_packet=False,
                        queue_num=nextsq())

            if PH < 5:
                dum = sb.tile([128, 1, D], F32)
                nc.vector.memset(dum[:].rearrange("p a b -> p (a b)"), 0.0)
                nc.sync.dma_start(
                    y_d[0:128, :].rearrange("(o p) c -> p o c", o=1), dum[:])
                return
            # ---- BN stats: stream h_a+h_b, keep sum resident ----
            u = sb.tile([128, VB, D], F32)
            scr = sb.tile([128, 16, D], F32)
            acc2 = sb.tile([128, 2 * 13, D], F32)
            sacc = sb.tile([128, 2 * D], F32)
            hav2 = h_a[:, :].rearrange("(p m) c -> p m c", p=128)
            hbv2 = h_b[:, :].rearrange("(p m) c -> p m c", p=128)
            for ci, b0 in enumerate(range(0, VB, 16)):
                nb = min(16, VB - b0)
                ta = cio.tile([128, 16, D], F32, tag="ta")
                nc.sync.dma_start(ta[:, 0:nb, :], hav2[:, b0:b0 + nb, :])
                tb = cio.tile([128, 16, D], F32, tag="tb")
                nc.sync.dma_start(tb[:, 0:nb, :], hbv2[:, b0:b0 + nb, :])
                nc.vector.tensor_tensor(
                    out=u[:, b0:b0 + nb, :], in0=ta[:, 0:nb, :],
                    in1=tb[:, 0:nb, :], op=mybir.AluOpType.add)
                nc.vector.tensor_reduce(
                    out=acc2[:, 2 * ci, :],
                    in_=u[:, b0:b0 + nb, :].transpose([0, 2, 1]),
                    axis=mybir.AxisListType.X, op=mybir.AluOpType.add)
                nc.vector.tensor_tensor(
                    out=scr[:, 0:nb, :], in0=u[:, b0:b0 + nb, :],
                    in1=u[:, b0:b0 + nb, :], op=mybir.AluOpType.mult)
                nc.vector.tensor_reduce(
                    out=acc2[:, 2 * ci + 1, :],
                    in_=scr[:, 0:nb, :].transpose([0, 2, 1]),
                    axis=mybir.AxisListType.X, op=mybir.AluOpType.add)
            nc.vector.tensor_reduce(
                out=sacc[:, 0:D],
                in_=acc2[:, 0:26:2, :].transpose([0, 2, 1]),
                axis=mybir.AxisListType.X, op=mybir.AluOpType.add)
            nc.vector.tensor_reduce(
                out=sacc[:, D:2 * D],
                in_=acc2[:, 1:26:2, :].transpose([0, 2, 1]),
                axis=mybir.AxisListType.X, op=mybir.AluOpType.add)

            cc_in = dram.tile([128, 2 * D], F32)
            cc_out = dram.tile([128, 2 * D], F32)
            nc.gpsimd.dma_start(cc_in[:], sacc[:])
            import os as _os
            if _os.environ.get("V2NOCC"):
                nc.gpsimd.dma_start(cc_out[:], cc_in[:])
            else:
                nc.gpsimd.collective_compute(
                    "AllReduce", mybir.AluOpType.add,
                    replica_groups=[list(range(C))],
                    ins=[cc_in.opt()], outs=[cc_out.opt()])
            tot = sb.tile([128, 2 * D], F32)
            nc.sync.dma_start(tot[:], cc_out[:])
            totr = sb.tile([128, 2 * D], F32)
            nc.gpsimd.partition_all_reduce(totr[:], tot[:], 128,
                                           bass_isa.ReduceOp.add)

            me = sb.tile([128, 2 * D], F32)
            nc.vector.tensor_scalar_mul(me[:], totr[:], 1.0 / N)
            var = sb.tile([128, D], F32)
            nc.vector.tensor_tensor(out=var[:], in0=me[:, 0:D],
                                    in1=me[:, 0:D], op=mybir.AluOpType.mult)
            nc.vector.tensor_tensor(out=var[:], in0=me[:, D:2 * D],
                                    in1=var[:], op=mybir.AluOpType.subtract)
            eps_t = sb.tile([128, 1], F32)
            nc.gpsimd.memset(eps_t[:], EPS)
            std = sb.tile([128, D], F32)
            nc.scalar.activation(std[:], var[:],
                                 mybir.ActivationFunctionType.Sqrt,
                                 bias=eps_t[:])
            rstd = sb.tile([128, D], F32)
            nc.vector.reciprocal(rstd[:], std[:])
            gam = sb.tile([128, D], F32)
            nc.sync.dma_start(gam[:], gam_d[:, :])
            bet = sb.tile([128, D], F32)
            nc.sync.dma_start(bet[:], bet_d[:, :])
            sc_f = sb.tile([128, D], F32)
            nc.vector.tensor_tensor(out=sc_f[:], in0=rstd[:], in1=gam[:],
                                    op=mybir.AluOpType.mult)
            cb = sb.tile([128, D], F32)
            nc.vector.tensor_tensor(out=cb[:], in0=me[:, 0:D], in1=sc_f[:],
                                    op=mybir.AluOpType.mult)
            nc.vector.tensor_tensor(out=cb[:], in0=bet[:], in1=cb[:],
                                    op=mybir.AluOpType.subtract)

            # ---- apply + y write (chunked so DVE/ACT/DMA pipeline) ----
            # u holds h sum; y row = 128m + p, m = u block index.
            yv = y_d[0:24960, :].rearrange("(m p) c -> p m c", p=128)
            APC = 28
            for a0 in range(0, VB, APC):
                na = min(APC, VB - a0)
                sc_bc = sc_f[:, None, :].to_broadcast([128, na, D])
                cb_bc = cb[:, None, :].to_broadcast([128, na, D])
                nc.vector.tensor_tensor(out=u[:, a0:a0 + na, :],
                                        in0=u[:, a0:a0 + na, :],
                                        in1=sc_bc, op=mybir.AluOpType.mult)
                nc.vector.tensor_tensor(out=u[:, a0:a0 + na, :],
                                        in0=u[:, a0:a0 + na, :],
                                        in1=cb_bc, op=mybir.AluOpType.add)
                nc.scalar.activation(u[:, a0:a0 + na, :], u[:, a0:a0 + na, :],
                                     mybir.ActivationFunctionType.Lrelu,
                                     bias=0.0, alpha=NEG)
                hi = min(a0 + na, 195)
                if hi > a0:
                    nc.sync.dma_start(yv[:, a0:hi, :], u[:, a0:hi, :])
            nc.sync.dma_start(
                y_d[24960:25000, :].rearrange("(o p) c -> p o c", o=1),
                u[0:40, 195:196, :])


_CACHE = {}


def build(nbr):
    nbr = np.asarray(nbr)
    key = nbr.tobytes()[:4096] + nbr.tobytes()[-4096:]
    if key in _CACHE:
        return _CACHE[key]
    plan, GT, gslab, sslab = _prep_host(nbr)
    nc = bacc.Bacc("TRN2", target_bir_lowering=False, debug=False,
                   num_devices=C, num_swdge_queues=4)
    _build_body(nc, plan, GT)
    nc.compile()
    _CACHE[key] = (nc, gslab, sslab)
    return nc, gslab, sslab


def _to_bf16(a):
    import jax.numpy as jnp
    return np.asarray(jnp.asarray(a, jnp.bfloat16))


def make_in_maps(x, W, gamma, beta, gslab, sslab):
    xpad = np.zeros((N + 8, D), np.float32)
    xpad[:N] = x
    x_bf = _to_bf16(xpad)
    w2 = np.zeros((128, K * D), np.float32)
    for k in range(K):
        w2[0:D, k * D:(k + 1) * D] = W[k]
    w2_bf = _to_bf16(w2)
    gam_r = np.ascontiguousarray(np.broadcast_to(gamma, (128, D)))
    bet_r = np.ascontiguousarray(np.broadcast_to(beta, (128, D)))
    in_maps = []
    for c in range(C):
        xc = np.zeros((D, VB * 128), np.float32)
        xc[:, 0:V] = x[c * V:(c + 1) * V].T
        in_maps.append({
            "x_d": x_bf,
            "xct_d": _to_bf16(xc),
            "w2_d": w2_bf,
            "gam_d": gam_r,
            "bet_d": bet_r,
            "gi_d": gslab[c],
            "si_d": sslab[c],
        })
    return in_maps


def kernel(x, W, gamma, beta, nbr):
    x = np.ascontiguousarray(np.asarray(x, np.float32))
    W = np.ascontiguousarray(np.asarray(W, np.float32))
    gamma = np.asarray(gamma, np.float32).reshape(D)
    beta = np.asarray(beta, np.float32).reshape(D)
    nbr = np.asarray(nbr)
    nc, gslab, sslab = build(nbr)
    in_maps = make_in_maps(x, W, gamma, beta, gslab, sslab)
    res = bass_utils.run_bass_kernel_spmd(nc, in_maps, core_ids=list(range(C)))
    return np.concatenate([res.results[c]["y_d"] for c in range(C)], axis=0)
